# revision 24
# baseline (speedup 1.0000x reference)
"""BFLinear (block-floating-point quantized linear) Trainium2 kernel.

Computes: out = bf_quant(bf_quant(x) @ bf_quant(W).T + 2*b)
where bf_quant quantizes groups of 32 along the last axis to a shared
power-of-two exponent with 8 mantissa bits (values = int8 * 2^(e-7)).

Distribution over 8 NeuronCores:
  - batch dim of x sharded 8 ways (1024 rows/core)
  - W quantization split by output rows (512 rows/core); the quantized
    slab is transposed to [in, out] layout on the PE array (identity
    matmuls) while the PE is otherwise idle, then AllGathered as bf16 in
    two 256-column halves so matmuls against the first half can start
    while the second AllGather is still in flight
  - x is quantized on-chip and PE-transposed into a resident SBUF
    [in, batch] buffer - no DRAM round trip
  - matmul runs in bf16 (quantized values are exact in bf16), fp32 PSUM
    accumulation, k-innermost; each PSUM bank holds TWO 256-column row
    blocks so drains amortize over [128, 512] tiles
  - 2*b enters via a K=2 all-ones matmul against [bh; bl] (bf16 high/low
    split of 2b, exact to ~1e-7) as the accumulation opener - this also
    sets PSUM has_written correctly
  - engine split for quantization: DVE does reduce/bit-math/final
    scale-multiply, GpSimd does the pre-round clamp + (+C) RNE round,
    ScalarE only does PSUM->SBUF copies

Quantization math (matching jnp semantics):
  m     = max |x| over each group of 32
  scale = 2^(floor(log2 m) - 7)        (exponent-field bit math)
  inv   = 1/scale                      (bit math, exact)
  v     = clamp(x*inv, -128.25, 127.25) (== post-round clip)
  r     = rne_round(v) via +C trick, C = 1.5*2^23
  q     = (r - C) * scale              (exact in bf16)
"""

import numpy as np

B_FULL = 8192
IN_FULL = 4096
OUT_FULL = 4096
NCORES = 8

P = 128
SZ = 32
C_RND = float(3 * 2**22)  # 1.5*2^23


def build_nc(b_sh=B_FULL // NCORES, in_dim=IN_FULL, out_dim=OUT_FULL,
             ncores=NCORES, for_timeline=False):
    """Build the SPMD Bass program (identical on every core; data differs)."""
    import concourse.mybir as mybir
    import concourse.tile as tile
    from concourse import bacc, masks

    F32 = mybir.dt.float32
    BF16 = mybir.dt.bfloat16
    FP16 = mybir.dt.float16
    I32 = mybir.dt.int32
    ALU = mybir.AluOpType
    AX = mybir.AxisListType
    AF = mybir.ActivationFunctionType

    w_sl = out_dim // ncores      # W rows quantized on this core (512)
    k_chunks = in_dim // P        # 128-wide contraction chunks (32)
    nbb = b_sh // P               # batch row blocks (8)
    half = w_sl // 2              # o-columns per AG half (256)

    nc = bacc.Bacc("TRN2", target_bir_lowering=False, debug=False,
                   num_devices=ncores)

    x_sh = nc.dram_tensor("x_sh", [b_sh, in_dim], F32, kind="ExternalInput")
    w_sl_t = nc.dram_tensor("w_sl", [w_sl, in_dim], F32, kind="ExternalInput")
    b2bf_d = nc.dram_tensor("b2bf", [2, out_dim], BF16, kind="ExternalInput")
    ones_d = nc.dram_tensor("ones2", [2, P], BF16, kind="ExternalInput")
    out_sh = nc.dram_tensor("out_sh", [b_sh, out_dim], BF16,
                            kind="ExternalOutput")

    wqT_lo = nc.dram_tensor("wqT_lo", [in_dim, half], BF16)
    wqT_hi = nc.dram_tensor("wqT_hi", [in_dim, half], BF16)
    ag_lo = nc.dram_tensor("ag_lo", [ncores * in_dim, half], BF16,
                           addr_space="Shared")
    ag_hi = nc.dram_tensor("ag_hi", [ncores * in_dim, half], BF16,
                           addr_space="Shared")

    with tile.TileContext(nc) as tc:
        from contextlib import ExitStack
        with ExitStack() as ctx:
            xpool = ctx.enter_context(tc.tile_pool(name="xpool", bufs=2))
            qpool = ctx.enter_context(tc.tile_pool(name="qpool", bufs=2))
            spool = ctx.enter_context(tc.tile_pool(name="spool", bufs=2))
            bpool = ctx.enter_context(tc.tile_pool(name="bpool", bufs=4))
            dpool = ctx.enter_context(tc.tile_pool(name="dpool", bufs=3))
            opool = ctx.enter_context(tc.tile_pool(name="opool", bufs=3))
            dsp = ctx.enter_context(tc.tile_pool(name="dsp", bufs=3))
            pmm = ctx.enter_context(
                tc.tile_pool(name="pmm", bufs=5, space="PSUM"))
            ptp = ctx.enter_context(
                tc.tile_pool(name="ptp", bufs=3, space="PSUM"))
            singles = ctx.enter_context(tc.tile_pool(name="singles", bufs=1))

            ident = singles.tile([P, P], BF16, tag="ident")
            masks.make_identity(nc, ident[:])
            ones2 = singles.tile([2, P], BF16, tag="ones2")
            nc.sync.dma_start(ones2[:], ones_d.ap())
            b2bf = singles.tile([2, out_dim], BF16, tag="b2bf")
            nc.sync.dma_start(b2bf[:], b2bf_d.ap())

            # resident transposed x
            xqT = singles.tile([P, k_chunks, b_sh], BF16, tag="xqT")

            def quant(t, width, q_out, sp, tagp, r_tile=None):
                """Quantize an SBUF-resident [P, width] tile into q_out
                (bf16 exact). t may be f32 (clobbered in place) or bf16
                (exact under power-of-two scaling); for bf16 input pass a
                f32 r_tile for the rounded intermediate."""
                g = width // SZ
                t3 = t.rearrange("p (g s) -> p g s", s=SZ)
                m = sp.tile([P, g], F32, tag=f"{tagp}_m")
                nc.vector.tensor_reduce(m[:], t3, axis=AX.X, op=ALU.max,
                                        apply_absolute_value=True)
                scale = sp.tile([P, g], F32, tag=f"{tagp}_scale")
                # scale_bits = (m_bits & 0x7F800000) - (7 << 23)
                nc.vector.tensor_scalar(
                    scale[:].bitcast(I32), m[:].bitcast(I32),
                    0x7F800000, None, op0=ALU.bitwise_and)
                nc.vector.tensor_scalar(
                    scale[:].bitcast(I32), scale[:].bitcast(I32),
                    7 << 23, None, op0=ALU.subtract)
                inv = sp.tile([P, g], F32, tag=f"{tagp}_inv")
                # inv_bits = (254<<23) - scale_bits
                nc.vector.tensor_scalar(
                    inv[:].bitcast(I32), scale[:].bitcast(I32),
                    -1, None, op0=ALU.bitwise_xor)
                nc.vector.tensor_scalar(
                    inv[:].bitcast(I32), inv[:].bitcast(I32),
                    (254 << 23) + 1, None, op0=ALU.add)
                # v = x * inv (exact power-of-two scaling) - DVE
                nc.vector.tensor_tensor(
                    t3, t3, inv[:, :, None].to_broadcast([P, g, SZ]),
                    ALU.mult)
                # pre-round clamp (== post-round clip; round is monotonic)
                nc.vector.tensor_scalar(
                    t, t, -128.25, 127.25, op0=ALU.max, op1=ALU.min)
                # +C forces RNE-to-integer on the scalar engine (must land
                # in an f32 tile; ulp=1 in [2^23, 2^24))
                r = t if r_tile is None else r_tile
                nc.scalar.activation(r, t, AF.Copy, bias=C_RND, scale=1.0)
                # q = (r - C) * scale, fused subtract+scale - DVE
                nc.vector.scalar_tensor_tensor(
                    q_out.rearrange("p (g s) -> p g s", s=SZ),
                    r.rearrange("p (g s) -> p g s", s=SZ), C_RND,
                    scale[:, :, None].to_broadcast([P, g, SZ]),
                    op0=ALU.subtract, op1=ALU.mult)

            def pe_transpose_into(src_bf16, dest, col_base):
                """PE-transpose [P, in_dim] bf16 src into dest[:, k,
                col_base:col_base+P]; 4 transposes share one PSUM bank so
                the ScalarE drain is one [P, 512] copy per 4."""
                for k4 in range(k_chunks // 4):
                    pst = ptp.tile([P, 4 * P], BF16, tag="pst",
                                   padded_shape=[P, 1024])
                    for i in range(4):
                        k = k4 * 4 + i
                        nc.tensor.matmul(
                            pst[:, i * P:(i + 1) * P],
                            lhsT=src_bf16[:, k * P:(k + 1) * P],
                            rhs=ident[:], is_transpose=True,
                            skip_group_check=True)
                    nc.scalar.copy(
                        dest[:, k4 * 4:k4 * 4 + 4, col_base:col_base + P],
                        pst[:].rearrange("p (a b) -> p a b", a=4))

            # ---- W: quantize + PE-transpose; each AG covers the W half
            # that feeds it, so AG1 launches after only 2 W tiles ----------
            def ag_half(src_dram, dst_shared):
                if for_timeline:
                    nc.sync.dma_start(dst_shared.ap()[0:in_dim, :],
                                      src_dram.ap())
                else:
                    nc.gpsimd.collective_compute(
                        "AllGather", ALU.bypass,
                        replica_groups=[list(range(ncores))],
                        ins=[src_dram.ap().opt()],
                        outs=[dst_shared.ap().opt()])

            with tc.tile_pool(name="wqtp", bufs=1) as wqtp:
                wqt = wqtp.tile([P, k_chunks, w_sl], BF16, tag="wqt")
                for hw, (wqT_d, ag_d) in enumerate(
                        ((wqT_lo, ag_lo), (wqT_hi, ag_hi))):
                    for r in (2 * hw, 2 * hw + 1):
                        wt = xpool.tile([P, in_dim], F32, tag="ld")
                        nc.sync.dma_start(wt[:],
                                          w_sl_t.ap()[r * P:(r + 1) * P, :])
                        wq = qpool.tile([P, in_dim], BF16, tag="q")
                        quant(wt[:], in_dim, wq[:], spool, "q")
                        pe_transpose_into(wq[:], wqt, r * P)
                    nc.gpsimd.dma_start(
                        wqT_d.ap().rearrange("(k p) o -> p k o", p=P),
                        wqt[:, :, hw * half:(hw + 1) * half])
                    ag_half(wqT_d, ag_d)

            # ---- x: f32 loads on the GpSimd queue (positioned after the
            # AG doorbells so nothing here can delay them; reduced-precision
            # x measurably breaks the 2e-2 gate - keep f32), quantize +
            # PE-transpose into xqT ---------------------------------------
            with tc.tile_pool(name="xfp", bufs=3) as xfp:
                for bb in range(nbb):
                    xt = xfp.tile([P, in_dim], F32, tag="xf")
                    nc.gpsimd.dma_start(xt[:],
                                        x_sh.ap()[bb * P:(bb + 1) * P, :])
                    xq = qpool.tile([P, in_dim], BF16, tag="q")
                    quant(xt[:], in_dim, xq[:], spool, "q")
                    pe_transpose_into(xq[:], xqT, bb * P)

            # slab prefetch pool reuses the released staging space
            wpool = ctx.enter_context(tc.tile_pool(name="wpool", bufs=4))

            unit_seq = [(h, ag, j) for h, ag in ((0, ag_lo), (1, ag_hi))
                        for j in range(ncores)]

            # slab loads ride the GpSimd queue behind the AG doorbells and
            # the x cast-loads; the first four fire the instant AG1's data
            # lands, wpool buffer rotation paces the rest
            slab_tiles = []
            for h, ag, j in unit_seq:
                slab = wpool.tile([P, k_chunks, half], BF16, tag="slab")
                nc.gpsimd.dma_start(
                    slab[:],
                    ag.ap()[j * in_dim:(j + 1) * in_dim, :]
                    .rearrange("(k p) o -> p k o", p=P))
                slab_tiles.append(slab)

            # ---- matmul waves: unit = (j, half). Units run in PAIRS with
            # pp (row-block pair) outermost inside the pair, so early pps
            # only need the first x row blocks - late-arriving x tiles do
            # not gate whole units.
            for pi in range(len(unit_seq) // 2):
                h, ag, _ = unit_seq[2 * pi]
                pair_js = (unit_seq[2 * pi][2], unit_seq[2 * pi + 1][2])
                slabs = slab_tiles[2 * pi:2 * pi + 2]
                b2us = []
                for j in pair_js:
                    col = j * w_sl + h * half
                    # bias rhs [2, 512]: the 256-col slice duplicated
                    b2u = bpool.tile([2, 2 * half], BF16, tag="b2u")
                    nc.vector.tensor_copy(b2u[:, 0:half],
                                          b2bf[:, col:col + half])
                    nc.vector.tensor_copy(b2u[:, half:2 * half],
                                          b2bf[:, col:col + half])
                    b2us.append(b2u)
                for pp in range(nbb // 2):
                    bb0, bb1 = 2 * pp, 2 * pp + 1
                    for ji, j in enumerate(pair_js):
                            col = j * w_sl + h * half
                            slab, b2u = slabs[ji], b2us[ji]
                            ps = pmm.tile([P, 2 * half], F32, tag="ps",
                                          padded_shape=[P, 512])
                            # bias opener: psum = [2b | 2b] via K=2 ones
                            # matmul; also sets has_written for the bank
                            nc.tensor.matmul(ps[:], lhsT=ones2[:],
                                             rhs=b2u[:], start=True,
                                             stop=False,
                                             skip_group_check=True)
                            for bx, bb in ((0, bb0), (1, bb1)):
                                for k in range(k_chunks):
                                    nc.tensor.matmul(
                                        ps[:, bx * half:(bx + 1) * half],
                                        lhsT=xqT[:, k, bb * P:(bb + 1) * P],
                                        rhs=slab[:, k, :],
                                        start=False,
                                        stop=(bx == 1 and
                                              k == k_chunks - 1),
                                        skip_group_check=True)
                            s = dpool.tile([P, 2 * half], F32, tag="s")
                            nc.vector.tensor_copy(s[:], ps[:])
                            oq = opool.tile([P, 2 * half], BF16, tag="oq")
                            quant(s[:], 2 * half, oq[:], dsp, "d")
                            nc.scalar.dma_start(
                                out_sh.ap()[bb0 * P:(bb0 + 1) * P,
                                            col:col + half],
                                oq[:, 0:half])
                            nc.scalar.dma_start(
                                out_sh.ap()[bb1 * P:(bb1 + 1) * P,
                                            col:col + half],
                                oq[:, half:2 * half])

    nc.compile()
    return nc


_NC_CACHE = {}


def _get_nc(key=(B_FULL // NCORES, IN_FULL, OUT_FULL, NCORES)):
    if key not in _NC_CACHE:
        _NC_CACHE[key] = build_nc(*key)
    return _NC_CACHE[key]


def make_in_maps(x, W, b, ncores=NCORES):
    import ml_dtypes
    b_sh = x.shape[0] // ncores
    w_sl = W.shape[0] // ncores
    out_dim = W.shape[0]
    b2 = 2.0 * np.asarray(b, np.float32)
    bh = b2.astype(ml_dtypes.bfloat16)
    bl = (b2 - bh.astype(np.float32)).astype(ml_dtypes.bfloat16)
    b2bf = np.ascontiguousarray(np.stack([bh, bl], axis=0))
    ones2 = np.ones((2, P), ml_dtypes.bfloat16)
    return [
        {
            "x_sh": np.ascontiguousarray(x[c * b_sh:(c + 1) * b_sh]),
            "w_sl": np.ascontiguousarray(W[c * w_sl:(c + 1) * w_sl]),
            "b2bf": b2bf,
            "ones2": ones2,
        }
        for c in range(ncores)
    ]


def kernel(x, W, b):
    from concourse.bass_utils import run_bass_kernel_spmd

    x = np.asarray(x, np.float32)
    W = np.asarray(W, np.float32)
    b = np.asarray(b, np.float32)
    nc = _get_nc()
    in_maps = make_in_maps(x, W, b)
    res = run_bass_kernel_spmd(nc, in_maps, core_ids=list(range(NCORES)))
    return np.concatenate(
        [np.asarray(res.results[c]["out_sh"]).astype(np.float32)
         for c in range(NCORES)], axis=0)


# revision 25
# speedup vs baseline: 1.0663x; 1.0663x over previous
"""BFLinear (block-floating-point quantized linear) Trainium2 kernel.

Computes: out = bf_quant(bf_quant(x) @ bf_quant(W).T + 2*b)
where bf_quant quantizes groups of 32 along the last axis to a shared
power-of-two exponent with 8 mantissa bits (values = int8 * 2^(e-7)).

Distribution over 8 NeuronCores:
  - batch dim of x sharded 8 ways (1024 rows/core)
  - W quantization split by output rows (512 rows/core); the quantized
    slab is transposed to [in, out] layout on the PE array (identity
    matmuls) while the PE is otherwise idle, then AllGathered as bf16 in
    two 256-column halves so matmuls against the first half can start
    while the second AllGather is still in flight
  - x is quantized on-chip and PE-transposed into a resident SBUF
    [in, batch] buffer - no DRAM round trip
  - matmul runs in bf16 (quantized values are exact in bf16), fp32 PSUM
    accumulation, k-innermost; each PSUM bank holds TWO 256-column row
    blocks so drains amortize over [128, 512] tiles
  - 2*b enters via a K=2 all-ones matmul against [bh; bl] (bf16 high/low
    split of 2b, exact to ~1e-7) as the accumulation opener - this also
    sets PSUM has_written correctly
  - engine split for quantization: DVE does reduce/bit-math/final
    scale-multiply, GpSimd does the pre-round clamp + (+C) RNE round,
    ScalarE only does PSUM->SBUF copies

Quantization math (matching jnp semantics):
  m     = max |x| over each group of 32
  scale = 2^(floor(log2 m) - 7)        (exponent-field bit math)
  inv   = 1/scale                      (bit math, exact)
  v     = clamp(x*inv, -128.25, 127.25) (== post-round clip)
  r     = rne_round(v) via +C trick, C = 1.5*2^23
  q     = (r - C) * scale              (exact in bf16)
"""

import numpy as np

B_FULL = 8192
IN_FULL = 4096
OUT_FULL = 4096
NCORES = 8

P = 128
SZ = 32
C_RND = float(3 * 2**22)  # 1.5*2^23


def build_nc(b_sh=B_FULL // NCORES, in_dim=IN_FULL, out_dim=OUT_FULL,
             ncores=NCORES, for_timeline=False):
    """Build the SPMD Bass program (identical on every core; data differs)."""
    import concourse.mybir as mybir
    import concourse.tile as tile
    from concourse import bacc, masks

    F32 = mybir.dt.float32
    BF16 = mybir.dt.bfloat16
    FP16 = mybir.dt.float16
    I32 = mybir.dt.int32
    ALU = mybir.AluOpType
    AX = mybir.AxisListType
    AF = mybir.ActivationFunctionType

    w_sl = out_dim // ncores      # W rows quantized on this core (512)
    k_chunks = in_dim // P        # 128-wide contraction chunks (32)
    nbb = b_sh // P               # batch row blocks (8)
    half = w_sl // 2              # o-columns per AG half (256)

    nc = bacc.Bacc("TRN2", target_bir_lowering=False, debug=False,
                   num_devices=ncores)

    x_sh = nc.dram_tensor("x_sh", [b_sh, in_dim], F32, kind="ExternalInput")
    w_sl_t = nc.dram_tensor("w_sl", [w_sl, in_dim], F32, kind="ExternalInput")
    b2bf_d = nc.dram_tensor("b2bf", [2, out_dim], BF16, kind="ExternalInput")
    ones_d = nc.dram_tensor("ones2", [2, P], BF16, kind="ExternalInput")
    out_sh = nc.dram_tensor("out_sh", [b_sh, out_dim], BF16,
                            kind="ExternalOutput")

    wqT_lo = nc.dram_tensor("wqT_lo", [in_dim, half], BF16)
    wqT_hi = nc.dram_tensor("wqT_hi", [in_dim, half], BF16)
    ag_lo = nc.dram_tensor("ag_lo", [ncores * in_dim, half], BF16,
                           addr_space="Shared")
    ag_hi = nc.dram_tensor("ag_hi", [ncores * in_dim, half], BF16,
                           addr_space="Shared")

    with tile.TileContext(nc) as tc:
        from contextlib import ExitStack
        with ExitStack() as ctx:
            xpool = ctx.enter_context(tc.tile_pool(name="xpool", bufs=2))
            qpool = ctx.enter_context(tc.tile_pool(name="qpool", bufs=2))
            spool = ctx.enter_context(tc.tile_pool(name="spool", bufs=2))
            bpool = ctx.enter_context(tc.tile_pool(name="bpool", bufs=4))
            dpool = ctx.enter_context(tc.tile_pool(name="dpool", bufs=3))
            opool = ctx.enter_context(tc.tile_pool(name="opool", bufs=3))
            dsp = ctx.enter_context(tc.tile_pool(name="dsp", bufs=3))
            pmm = ctx.enter_context(
                tc.tile_pool(name="pmm", bufs=5, space="PSUM"))
            ptp = ctx.enter_context(
                tc.tile_pool(name="ptp", bufs=3, space="PSUM"))
            singles = ctx.enter_context(tc.tile_pool(name="singles", bufs=1))

            ident = singles.tile([P, P], BF16, tag="ident")
            masks.make_identity(nc, ident[:])
            ones2 = singles.tile([2, P], BF16, tag="ones2")
            nc.sync.dma_start(ones2[:], ones_d.ap())
            b2bf = singles.tile([2, out_dim], BF16, tag="b2bf")
            nc.sync.dma_start(b2bf[:], b2bf_d.ap())

            # resident transposed x
            xqT = singles.tile([P, k_chunks, b_sh], BF16, tag="xqT")

            def quant(t, width, q_out, sp, tagp, r_tile=None):
                """Quantize an SBUF-resident [P, width] tile into q_out
                (bf16 exact). t may be f32 (clobbered in place) or bf16
                (exact under power-of-two scaling); for bf16 input pass a
                f32 r_tile for the rounded intermediate."""
                g = width // SZ
                t3 = t.rearrange("p (g s) -> p g s", s=SZ)
                m = sp.tile([P, g], F32, tag=f"{tagp}_m")
                nc.vector.tensor_reduce(m[:], t3, axis=AX.X, op=ALU.max,
                                        apply_absolute_value=True)
                scale = sp.tile([P, g], F32, tag=f"{tagp}_scale")
                # scale_bits = (m_bits & 0x7F800000) - (7 << 23)
                nc.vector.tensor_scalar(
                    scale[:].bitcast(I32), m[:].bitcast(I32),
                    0x7F800000, None, op0=ALU.bitwise_and)
                nc.vector.tensor_scalar(
                    scale[:].bitcast(I32), scale[:].bitcast(I32),
                    7 << 23, None, op0=ALU.subtract)
                inv = sp.tile([P, g], F32, tag=f"{tagp}_inv")
                # inv_bits = (254<<23) - scale_bits
                nc.vector.tensor_scalar(
                    inv[:].bitcast(I32), scale[:].bitcast(I32),
                    -1, None, op0=ALU.bitwise_xor)
                nc.vector.tensor_scalar(
                    inv[:].bitcast(I32), inv[:].bitcast(I32),
                    (254 << 23) + 1, None, op0=ALU.add)
                # v = x * inv (exact power-of-two scaling) - DVE
                nc.vector.tensor_tensor(
                    t3, t3, inv[:, :, None].to_broadcast([P, g, SZ]),
                    ALU.mult)
                # pre-round clamp (== post-round clip; round is monotonic)
                nc.vector.tensor_scalar(
                    t, t, -128.25, 127.25, op0=ALU.max, op1=ALU.min)
                # +C forces RNE-to-integer on the scalar engine (must land
                # in an f32 tile; ulp=1 in [2^23, 2^24))
                r = t if r_tile is None else r_tile
                nc.scalar.activation(r, t, AF.Copy, bias=C_RND, scale=1.0)
                # q = (r - C) * scale, fused subtract+scale - DVE
                nc.vector.scalar_tensor_tensor(
                    q_out.rearrange("p (g s) -> p g s", s=SZ),
                    r.rearrange("p (g s) -> p g s", s=SZ), C_RND,
                    scale[:, :, None].to_broadcast([P, g, SZ]),
                    op0=ALU.subtract, op1=ALU.mult)

            def pe_transpose_into(src_bf16, dest, col_base):
                """PE-transpose [P, in_dim] bf16 src into dest[:, k,
                col_base:col_base+P]; 4 transposes share one PSUM bank so
                the ScalarE drain is one [P, 512] copy per 4."""
                for k4 in range(k_chunks // 4):
                    pst = ptp.tile([P, 4 * P], BF16, tag="pst",
                                   padded_shape=[P, 1024])
                    for i in range(4):
                        k = k4 * 4 + i
                        nc.tensor.matmul(
                            pst[:, i * P:(i + 1) * P],
                            lhsT=src_bf16[:, k * P:(k + 1) * P],
                            rhs=ident[:], is_transpose=True,
                            skip_group_check=True)
                    nc.scalar.copy(
                        dest[:, k4 * 4:k4 * 4 + 4, col_base:col_base + P],
                        pst[:].rearrange("p (a b) -> p a b", a=4))

            # ---- W: quantize + PE-transpose; each AG covers the W half
            # that feeds it, so AG1 launches after only 2 W tiles ----------
            def ag_half(src_dram, dst_shared):
                if for_timeline:
                    nc.sync.dma_start(dst_shared.ap()[0:in_dim, :],
                                      src_dram.ap())
                else:
                    nc.gpsimd.collective_compute(
                        "AllGather", ALU.bypass,
                        replica_groups=[list(range(ncores))],
                        ins=[src_dram.ap().opt()],
                        outs=[dst_shared.ap().opt()])

            with tc.tile_pool(name="wqtp", bufs=1) as wqtp:
                wqt = wqtp.tile([P, k_chunks, w_sl], BF16, tag="wqt")
                for hw, (wqT_d, ag_d) in enumerate(
                        ((wqT_lo, ag_lo), (wqT_hi, ag_hi))):
                    for r in (2 * hw, 2 * hw + 1):
                        wt = xpool.tile([P, in_dim], F32, tag="ld")
                        nc.sync.dma_start(wt[:],
                                          w_sl_t.ap()[r * P:(r + 1) * P, :])
                        wq = qpool.tile([P, in_dim], BF16, tag="q")
                        quant(wt[:], in_dim, wq[:], spool, "q")
                        pe_transpose_into(wq[:], wqt, r * P)
                    nc.gpsimd.dma_start(
                        wqT_d.ap().rearrange("(k p) o -> p k o", p=P),
                        wqt[:, :, hw * half:(hw + 1) * half])
                    ag_half(wqT_d, ag_d)

            # ---- x: f32 loads on the GpSimd queue (positioned after the
            # AG doorbells so nothing here can delay them; reduced-precision
            # x measurably breaks the 2e-2 gate - keep f32), quantize +
            # PE-transpose into xqT ---------------------------------------
            with tc.tile_pool(name="xfp", bufs=3) as xfp:
                for bb in range(nbb):
                    xt = xfp.tile([P, in_dim], F32, tag="xf")
                    nc.sync.dma_start(xt[:],
                                      x_sh.ap()[bb * P:(bb + 1) * P, :])
                    xq = qpool.tile([P, in_dim], BF16, tag="q")
                    quant(xt[:], in_dim, xq[:], spool, "q")
                    pe_transpose_into(xq[:], xqT, bb * P)

            # slab prefetch pool reuses the released staging space
            wpool = ctx.enter_context(tc.tile_pool(name="wpool", bufs=4))

            unit_seq = [(h, ag, j) for h, ag in ((0, ag_lo), (1, ag_hi))
                        for j in range(ncores)]

            # slab loads ride the GpSimd queue behind the AG doorbells and
            # the x cast-loads; the first four fire the instant AG1's data
            # lands, wpool buffer rotation paces the rest
            slab_tiles = []
            for h, ag, j in unit_seq:
                slab = wpool.tile([P, k_chunks, half], BF16, tag="slab")
                nc.gpsimd.dma_start(
                    slab[:],
                    ag.ap()[j * in_dim:(j + 1) * in_dim, :]
                    .rearrange("(k p) o -> p k o", p=P))
                slab_tiles.append(slab)

            # ---- matmul waves: unit = (j, half). Units run in PAIRS with
            # pp (row-block pair) outermost inside the pair, so early pps
            # only need the first x row blocks - late-arriving x tiles do
            # not gate whole units.
            for pi in range(len(unit_seq) // 2):
                h, ag, _ = unit_seq[2 * pi]
                pair_js = (unit_seq[2 * pi][2], unit_seq[2 * pi + 1][2])
                slabs = slab_tiles[2 * pi:2 * pi + 2]
                b2us = []
                for j in pair_js:
                    col = j * w_sl + h * half
                    # bias rhs [2, 512]: the 256-col slice duplicated
                    b2u = bpool.tile([2, 2 * half], BF16, tag="b2u")
                    nc.vector.tensor_copy(b2u[:, 0:half],
                                          b2bf[:, col:col + half])
                    nc.vector.tensor_copy(b2u[:, half:2 * half],
                                          b2bf[:, col:col + half])
                    b2us.append(b2u)
                for pp in range(nbb // 2):
                    bb0, bb1 = 2 * pp, 2 * pp + 1
                    for ji, j in enumerate(pair_js):
                            col = j * w_sl + h * half
                            slab, b2u = slabs[ji], b2us[ji]
                            ps = pmm.tile([P, 2 * half], F32, tag="ps",
                                          padded_shape=[P, 512])
                            # bias opener: psum = [2b | 2b] via K=2 ones
                            # matmul; also sets has_written for the bank
                            nc.tensor.matmul(ps[:], lhsT=ones2[:],
                                             rhs=b2u[:], start=True,
                                             stop=False,
                                             skip_group_check=True)
                            for bx, bb in ((0, bb0), (1, bb1)):
                                for k in range(k_chunks):
                                    nc.tensor.matmul(
                                        ps[:, bx * half:(bx + 1) * half],
                                        lhsT=xqT[:, k, bb * P:(bb + 1) * P],
                                        rhs=slab[:, k, :],
                                        start=False,
                                        stop=(bx == 1 and
                                              k == k_chunks - 1),
                                        skip_group_check=True)
                            s = dpool.tile([P, 2 * half], F32, tag="s")
                            nc.vector.tensor_copy(s[:], ps[:])
                            oq = opool.tile([P, 2 * half], BF16, tag="oq")
                            quant(s[:], 2 * half, oq[:], dsp, "d")
                            nc.scalar.dma_start(
                                out_sh.ap()[bb0 * P:(bb0 + 1) * P,
                                            col:col + half],
                                oq[:, 0:half])
                            nc.scalar.dma_start(
                                out_sh.ap()[bb1 * P:(bb1 + 1) * P,
                                            col:col + half],
                                oq[:, half:2 * half])

    nc.compile()
    return nc


_NC_CACHE = {}


def _get_nc(key=(B_FULL // NCORES, IN_FULL, OUT_FULL, NCORES)):
    if key not in _NC_CACHE:
        _NC_CACHE[key] = build_nc(*key)
    return _NC_CACHE[key]


def make_in_maps(x, W, b, ncores=NCORES):
    import ml_dtypes
    b_sh = x.shape[0] // ncores
    w_sl = W.shape[0] // ncores
    out_dim = W.shape[0]
    b2 = 2.0 * np.asarray(b, np.float32)
    bh = b2.astype(ml_dtypes.bfloat16)
    bl = (b2 - bh.astype(np.float32)).astype(ml_dtypes.bfloat16)
    b2bf = np.ascontiguousarray(np.stack([bh, bl], axis=0))
    ones2 = np.ones((2, P), ml_dtypes.bfloat16)
    return [
        {
            "x_sh": np.ascontiguousarray(x[c * b_sh:(c + 1) * b_sh]),
            "w_sl": np.ascontiguousarray(W[c * w_sl:(c + 1) * w_sl]),
            "b2bf": b2bf,
            "ones2": ones2,
        }
        for c in range(ncores)
    ]


def kernel(x, W, b):
    from concourse.bass_utils import run_bass_kernel_spmd

    x = np.asarray(x, np.float32)
    W = np.asarray(W, np.float32)
    b = np.asarray(b, np.float32)
    nc = _get_nc()
    in_maps = make_in_maps(x, W, b)
    res = run_bass_kernel_spmd(nc, in_maps, core_ids=list(range(NCORES)))
    return np.concatenate(
        [np.asarray(res.results[c]["out_sh"]).astype(np.float32)
         for c in range(NCORES)], axis=0)


# revision 26
# speedup vs baseline: 1.0844x; 1.0170x over previous
"""BFLinear (block-floating-point quantized linear) Trainium2 kernel.

Computes: out = bf_quant(bf_quant(x) @ bf_quant(W).T + 2*b)
where bf_quant quantizes groups of 32 along the last axis to a shared
power-of-two exponent with 8 mantissa bits (values = int8 * 2^(e-7)).

Distribution over 8 NeuronCores:
  - batch dim of x sharded 8 ways (1024 rows/core)
  - W quantization split by output rows (512 rows/core); the quantized
    slab is transposed to [in, out] layout on the PE array (identity
    matmuls) while the PE is otherwise idle, then AllGathered as bf16 in
    two 256-column halves so matmuls against the first half can start
    while the second AllGather is still in flight
  - x is quantized on-chip and PE-transposed into a resident SBUF
    [in, batch] buffer - no DRAM round trip
  - matmul runs in bf16 (quantized values are exact in bf16), fp32 PSUM
    accumulation, k-innermost; each PSUM bank holds TWO 256-column row
    blocks so drains amortize over [128, 512] tiles
  - 2*b enters via a K=2 all-ones matmul against [bh; bl] (bf16 high/low
    split of 2b, exact to ~1e-7) as the accumulation opener - this also
    sets PSUM has_written correctly
  - engine/queue split so no collective-completion wait ever sits in
    front of pipeline work on a strict-FIFO queue: Sync carries W/x
    loads, GpSimd carries the wqT stores + AG doorbells + slab loads,
    ScalarE carries the +C rounding activations, transpose-drain copies
    and output stores, DVE carries the quantization arithmetic and the
    PSUM drain copies

Quantization math (matching jnp semantics):
  m     = max |x| over each group of 32
  scale = 2^(floor(log2 m) - 7)        (exponent-field bit math)
  inv   = 1/scale                      (bit math, exact)
  v     = clamp(x*inv, -128.25, 127.25) (== post-round clip)
  r     = rne_round(v) via +C trick, C = 1.5*2^23
  q     = (r - C) * scale              (exact in bf16)
"""

import numpy as np

B_FULL = 8192
IN_FULL = 4096
OUT_FULL = 4096
NCORES = 8

P = 128
SZ = 32
C_RND = float(3 * 2**22)  # 1.5*2^23


def build_nc(b_sh=B_FULL // NCORES, in_dim=IN_FULL, out_dim=OUT_FULL,
             ncores=NCORES, for_timeline=False):
    """Build the SPMD Bass program (identical on every core; data differs)."""
    import concourse.mybir as mybir
    import concourse.tile as tile
    from concourse import bacc, masks

    F32 = mybir.dt.float32
    BF16 = mybir.dt.bfloat16
    FP16 = mybir.dt.float16
    I32 = mybir.dt.int32
    ALU = mybir.AluOpType
    AX = mybir.AxisListType
    AF = mybir.ActivationFunctionType

    w_sl = out_dim // ncores      # W rows quantized on this core (512)
    k_chunks = in_dim // P        # 128-wide contraction chunks (32)
    nbb = b_sh // P               # batch row blocks (8)
    half = w_sl // 2              # o-columns per AG half (256)

    nc = bacc.Bacc("TRN2", target_bir_lowering=False, debug=False,
                   num_devices=ncores)

    x_sh = nc.dram_tensor("x_sh", [b_sh, in_dim], F32, kind="ExternalInput")
    w_sl_t = nc.dram_tensor("w_sl", [w_sl, in_dim], F32, kind="ExternalInput")
    b2bf_d = nc.dram_tensor("b2bf", [2, out_dim], BF16, kind="ExternalInput")
    ones_d = nc.dram_tensor("ones2", [2, P], BF16, kind="ExternalInput")
    out_sh = nc.dram_tensor("out_sh", [b_sh, out_dim], BF16,
                            kind="ExternalOutput")

    wqT_lo = nc.dram_tensor("wqT_lo", [in_dim, half], BF16)
    wqT_hi = nc.dram_tensor("wqT_hi", [in_dim, half], BF16)
    ag_lo = nc.dram_tensor("ag_lo", [ncores * in_dim, half], BF16,
                           addr_space="Shared")
    ag_hi = nc.dram_tensor("ag_hi", [ncores * in_dim, half], BF16,
                           addr_space="Shared")

    with tile.TileContext(nc) as tc:
        from contextlib import ExitStack
        with ExitStack() as ctx:
            xpool = ctx.enter_context(tc.tile_pool(name="xpool", bufs=2))
            qpool = ctx.enter_context(tc.tile_pool(name="qpool", bufs=2))
            spool = ctx.enter_context(tc.tile_pool(name="spool", bufs=2))
            bpool = ctx.enter_context(tc.tile_pool(name="bpool", bufs=4))
            dpool = ctx.enter_context(tc.tile_pool(name="dpool", bufs=3))
            opool = ctx.enter_context(tc.tile_pool(name="opool", bufs=3))
            dsp = ctx.enter_context(tc.tile_pool(name="dsp", bufs=3))
            pmm = ctx.enter_context(
                tc.tile_pool(name="pmm", bufs=5, space="PSUM"))
            ptp = ctx.enter_context(
                tc.tile_pool(name="ptp", bufs=3, space="PSUM"))
            singles = ctx.enter_context(tc.tile_pool(name="singles", bufs=1))

            ident = singles.tile([P, P], BF16, tag="ident")
            masks.make_identity(nc, ident[:])
            ones2 = singles.tile([2, P], BF16, tag="ones2")
            nc.sync.dma_start(ones2[:], ones_d.ap())
            b2bf = singles.tile([2, out_dim], BF16, tag="b2bf")
            nc.sync.dma_start(b2bf[:], b2bf_d.ap())

            # resident transposed x
            xqT = singles.tile([P, k_chunks, b_sh], BF16, tag="xqT")

            def quant(t, width, q_out, sp, tagp, r_tile=None):
                """Quantize an SBUF-resident [P, width] tile into q_out
                (bf16 exact). t may be f32 (clobbered in place) or bf16
                (exact under power-of-two scaling); for bf16 input pass a
                f32 r_tile for the rounded intermediate."""
                g = width // SZ
                t3 = t.rearrange("p (g s) -> p g s", s=SZ)
                m = sp.tile([P, g], F32, tag=f"{tagp}_m")
                nc.vector.tensor_reduce(m[:], t3, axis=AX.X, op=ALU.max,
                                        apply_absolute_value=True)
                scale = sp.tile([P, g], F32, tag=f"{tagp}_scale")
                # scale_bits = (m_bits & 0x7F800000) - (7 << 23)
                nc.vector.tensor_scalar(
                    scale[:].bitcast(I32), m[:].bitcast(I32),
                    0x7F800000, None, op0=ALU.bitwise_and)
                nc.vector.tensor_scalar(
                    scale[:].bitcast(I32), scale[:].bitcast(I32),
                    7 << 23, None, op0=ALU.subtract)
                inv = sp.tile([P, g], F32, tag=f"{tagp}_inv")
                # inv_bits = (254<<23) - scale_bits
                nc.vector.tensor_scalar(
                    inv[:].bitcast(I32), scale[:].bitcast(I32),
                    -1, None, op0=ALU.bitwise_xor)
                nc.vector.tensor_scalar(
                    inv[:].bitcast(I32), inv[:].bitcast(I32),
                    (254 << 23) + 1, None, op0=ALU.add)
                # v = x * inv (exact power-of-two scaling) - DVE
                nc.vector.tensor_tensor(
                    t3, t3, inv[:, :, None].to_broadcast([P, g, SZ]),
                    ALU.mult)
                # pre-round clamp (== post-round clip; round is monotonic)
                nc.vector.tensor_scalar(
                    t, t, -128.25, 127.25, op0=ALU.max, op1=ALU.min)
                # +C forces RNE-to-integer on the scalar engine (must land
                # in an f32 tile; ulp=1 in [2^23, 2^24))
                r = t if r_tile is None else r_tile
                nc.scalar.activation(r, t, AF.Copy, bias=C_RND, scale=1.0)
                # q = (r - C) * scale, fused subtract+scale - DVE
                nc.vector.scalar_tensor_tensor(
                    q_out.rearrange("p (g s) -> p g s", s=SZ),
                    r.rearrange("p (g s) -> p g s", s=SZ), C_RND,
                    scale[:, :, None].to_broadcast([P, g, SZ]),
                    op0=ALU.subtract, op1=ALU.mult)

            def pe_transpose_into(src_bf16, dest, col_base):
                """PE-transpose [P, in_dim] bf16 src into dest[:, k,
                col_base:col_base+P]; 4 transposes share one PSUM bank so
                the ScalarE drain is one [P, 512] copy per 4."""
                for k4 in range(k_chunks // 4):
                    pst = ptp.tile([P, 4 * P], BF16, tag="pst",
                                   padded_shape=[P, 1024])
                    for i in range(4):
                        k = k4 * 4 + i
                        nc.tensor.matmul(
                            pst[:, i * P:(i + 1) * P],
                            lhsT=src_bf16[:, k * P:(k + 1) * P],
                            rhs=ident[:], is_transpose=True,
                            skip_group_check=True)
                    nc.scalar.copy(
                        dest[:, k4 * 4:k4 * 4 + 4, col_base:col_base + P],
                        pst[:].rearrange("p (a b) -> p a b", a=4))

            # ---- W: quantize + PE-transpose; each AG covers the W half
            # that feeds it, so AG1 launches after only 2 W tiles ----------
            def ag_half(src_dram, dst_shared):
                if for_timeline:
                    nc.sync.dma_start(dst_shared.ap()[0:in_dim, :],
                                      src_dram.ap())
                else:
                    nc.gpsimd.collective_compute(
                        "AllGather", ALU.bypass,
                        replica_groups=[list(range(ncores))],
                        ins=[src_dram.ap().opt()],
                        outs=[dst_shared.ap().opt()])

            with tc.tile_pool(name="wqtp", bufs=1) as wqtp:
                wqt = wqtp.tile([P, k_chunks, w_sl], BF16, tag="wqt")
                for hw, (wqT_d, ag_d) in enumerate(
                        ((wqT_lo, ag_lo), (wqT_hi, ag_hi))):
                    for r in (2 * hw, 2 * hw + 1):
                        wt = xpool.tile([P, in_dim], F32, tag="ld")
                        nc.sync.dma_start(wt[:],
                                          w_sl_t.ap()[r * P:(r + 1) * P, :])
                        wq = qpool.tile([P, in_dim], BF16, tag="q")
                        quant(wt[:], in_dim, wq[:], spool, "q")
                        pe_transpose_into(wq[:], wqt, r * P)
                    nc.gpsimd.dma_start(
                        wqT_d.ap().rearrange("(k p) o -> p k o", p=P),
                        wqt[:, :, hw * half:(hw + 1) * half])
                    ag_half(wqT_d, ag_d)

            # ---- x: f32 loads on the GpSimd queue (positioned after the
            # AG doorbells so nothing here can delay them; reduced-precision
            # x measurably breaks the 2e-2 gate - keep f32), quantize +
            # PE-transpose into xqT ---------------------------------------
            with tc.tile_pool(name="xfp", bufs=3) as xfp:
                for bb in range(nbb):
                    xt = xfp.tile([P, in_dim], F32, tag="xf")
                    nc.sync.dma_start(xt[:],
                                      x_sh.ap()[bb * P:(bb + 1) * P, :])
                    xq = qpool.tile([P, in_dim], BF16, tag="q")
                    quant(xt[:], in_dim, xq[:], spool, "q")
                    pe_transpose_into(xq[:], xqT, bb * P)

            # slab prefetch pool reuses the released staging space
            wpool = ctx.enter_context(tc.tile_pool(name="wpool", bufs=4))

            unit_seq = [(h, ag, j) for h, ag in ((0, ag_lo), (1, ag_hi))
                        for j in range(ncores)]

            # slab loads ride the GpSimd queue behind the AG doorbells and
            # the x cast-loads; the first four fire the instant AG1's data
            # lands, wpool buffer rotation paces the rest
            slab_tiles = []
            for h, ag, j in unit_seq:
                slab = wpool.tile([P, k_chunks, half], BF16, tag="slab")
                nc.gpsimd.dma_start(
                    slab[:],
                    ag.ap()[j * in_dim:(j + 1) * in_dim, :]
                    .rearrange("(k p) o -> p k o", p=P))
                slab_tiles.append(slab)

            # ---- matmul waves: unit = (j, half). Units run in PAIRS with
            # pp (row-block pair) outermost inside the pair, so early pps
            # only need the first x row blocks - late-arriving x tiles do
            # not gate whole units.
            for pi in range(len(unit_seq) // 2):
                h, ag, _ = unit_seq[2 * pi]
                pair_js = (unit_seq[2 * pi][2], unit_seq[2 * pi + 1][2])
                slabs = slab_tiles[2 * pi:2 * pi + 2]
                b2us = []
                for j in pair_js:
                    col = j * w_sl + h * half
                    # bias rhs [2, 512]: the 256-col slice duplicated
                    b2u = bpool.tile([2, 2 * half], BF16, tag="b2u")
                    nc.vector.tensor_copy(b2u[:, 0:half],
                                          b2bf[:, col:col + half])
                    nc.vector.tensor_copy(b2u[:, half:2 * half],
                                          b2bf[:, col:col + half])
                    b2us.append(b2u)
                for pp in range(nbb // 2):
                    bb0, bb1 = 2 * pp, 2 * pp + 1
                    for ji, j in enumerate(pair_js):
                            col = j * w_sl + h * half
                            slab, b2u = slabs[ji], b2us[ji]
                            ps = pmm.tile([P, 2 * half], F32, tag="ps",
                                          padded_shape=[P, 512])
                            # bias opener: psum = [2b | 2b] via K=2 ones
                            # matmul; also sets has_written for the bank
                            nc.tensor.matmul(ps[:], lhsT=ones2[:],
                                             rhs=b2u[:], start=True,
                                             stop=False,
                                             skip_group_check=True)
                            for bx, bb in ((0, bb0), (1, bb1)):
                                for k in range(k_chunks):
                                    nc.tensor.matmul(
                                        ps[:, bx * half:(bx + 1) * half],
                                        lhsT=xqT[:, k, bb * P:(bb + 1) * P],
                                        rhs=slab[:, k, :],
                                        start=False,
                                        stop=(bx == 1 and
                                              k == k_chunks - 1),
                                        skip_group_check=True)
                            s = dpool.tile([P, 2 * half], F32, tag="s")
                            nc.vector.tensor_copy(s[:], ps[:])
                            oq = opool.tile([P, 2 * half], BF16, tag="oq")
                            quant(s[:], 2 * half, oq[:], dsp, "d")
                            nc.scalar.dma_start(
                                out_sh.ap()[bb0 * P:(bb0 + 1) * P,
                                            col:col + half],
                                oq[:, 0:half])
                            nc.scalar.dma_start(
                                out_sh.ap()[bb1 * P:(bb1 + 1) * P,
                                            col:col + half],
                                oq[:, half:2 * half])

    nc.compile()
    return nc


_NC_CACHE = {}


def _get_nc(key=(B_FULL // NCORES, IN_FULL, OUT_FULL, NCORES)):
    if key not in _NC_CACHE:
        _NC_CACHE[key] = build_nc(*key)
    return _NC_CACHE[key]


def make_in_maps(x, W, b, ncores=NCORES):
    import ml_dtypes
    b_sh = x.shape[0] // ncores
    w_sl = W.shape[0] // ncores
    out_dim = W.shape[0]
    b2 = 2.0 * np.asarray(b, np.float32)
    bh = b2.astype(ml_dtypes.bfloat16)
    bl = (b2 - bh.astype(np.float32)).astype(ml_dtypes.bfloat16)
    b2bf = np.ascontiguousarray(np.stack([bh, bl], axis=0))
    ones2 = np.ones((2, P), ml_dtypes.bfloat16)
    return [
        {
            "x_sh": np.ascontiguousarray(x[c * b_sh:(c + 1) * b_sh]),
            "w_sl": np.ascontiguousarray(W[c * w_sl:(c + 1) * w_sl]),
            "b2bf": b2bf,
            "ones2": ones2,
        }
        for c in range(ncores)
    ]


def kernel(x, W, b):
    from concourse.bass_utils import run_bass_kernel_spmd

    x = np.asarray(x, np.float32)
    W = np.asarray(W, np.float32)
    b = np.asarray(b, np.float32)
    nc = _get_nc()
    in_maps = make_in_maps(x, W, b)
    res = run_bass_kernel_spmd(nc, in_maps, core_ids=list(range(NCORES)))
    return np.concatenate(
        [np.asarray(res.results[c]["out_sh"]).astype(np.float32)
         for c in range(NCORES)], axis=0)
